# revision 7
# baseline (speedup 1.0000x reference)
"""GCN encoder (3-layer, PyG GCNConv normalize=False + BN eval + ReLU) on 8 trn2 cores.

v2 strategy (node/dst-sharded, graph-parallel, reassociated):
  - agg_l = A @ (h W^T) is computed as (A @ h) W^T: per dst tile, gather h rows
    for the tile's edges (dma_gather, int16 idx, lo/hi table split), aggregate
    with 128-edge one-hot matmul chains (one-hot scatter matrices generated
    ON-CHIP on DVE via iota/is_equal/mult from compact dslot/weight vectors),
    then apply the dense layer per tile (DVE transpose + 2 matmuls) with the
    BN scale folded into W and the bias added via a ones-row matmul chunk.
  - Layer 1 gathers directly from a replicated x table (no z1 AllGather).
  - h1/h2 tables are AllGathered in 4 chunks, overlapped with tile compute.
  - Padding lanes use negative gather indices (skipped by the DMA) and
    zero weights in the on-chip scatter matrices.
"""

import math
from dataclasses import dataclass

import ml_dtypes
import numpy as np

P = 128
HIDDEN = 256
CIN = 128

# feature flags (conservative=False uses baseline-proven idioms)
NEG_IDX = False        # pad gather idx with -1 (DMA skips) vs 0 (gathers row 0)
USE_SCALAR_ENG = False # psum->sbuf copies + relu on scalar engine vs DVE
USE_VEC_TRANSPOSE = False  # DVE transpose vs PE transpose
NCHUNKS = 1            # AllGather chunks per layer boundary


@dataclass
class Cfg:
    n: int = 50000
    e: int = 1600000
    ncores: int = 8
    tiles: int = 49

    @property
    def slots_per_core(self) -> int:
        return self.tiles * P

    @property
    def total_slots(self) -> int:
        return self.ncores * self.slots_per_core


CFG = Cfg()

# tile positions per AG chunk (sums to 49)
if NCHUNKS == 4:
    CHUNK_TILES = [13, 12, 12, 12]
elif NCHUNKS == 2:
    CHUNK_TILES = [25, 24]
else:
    CHUNK_TILES = [49]
CHUNK_P0 = [0]
for _t in CHUNK_TILES[:-1]:
    CHUNK_P0.append(CHUNK_P0[-1] + _t)
# global row base of each chunk region in the full table
CHUNK_BASE = [0]
for _t in CHUNK_TILES:
    CHUNK_BASE.append(CHUNK_BASE[-1] + 8 * P * _t)
# lo/hi gather-table boundary; must be < 32768 and at a chunk/core boundary
if NCHUNKS == 1:
    HALF_ROWS = 4 * P * 49  # core 0-3 vs 4-7 (rank-major layout), 25088
else:
    HALF_ROWS = CHUNK_BASE[2]


def _chunk_of_pos(p: int) -> int:
    for c in range(NCHUNKS - 1, -1, -1):
        if p >= CHUNK_P0[c]:
            return c
    return 0


# ---------------------------------------------------------------------------
# Host-side preprocessing
# ---------------------------------------------------------------------------

def _balance_nodes(indeg: np.ndarray, cfg: Cfg) -> tuple[np.ndarray, np.ndarray]:
    """Assign nodes to (bin = core*tiles + tile, lane) balancing per-bin indeg.
    Returns bin_of[n], lane_of[n]."""
    import heapq

    nbins = cfg.ncores * cfg.tiles
    order = np.argsort(-indeg, kind="stable")
    heap = [(0, b) for b in range(nbins)]
    heapq.heapify(heap)
    counts = np.zeros(nbins, dtype=np.int64)
    bin_of = np.empty(cfg.n, dtype=np.int64)
    lane_of = np.empty(cfg.n, dtype=np.int64)
    for v in order:
        load, b = heapq.heappop(heap)
        bin_of[v] = b
        lane_of[v] = counts[b]
        counts[b] += 1
        load += int(indeg[v])
        if counts[b] < P:
            heapq.heappush(heap, (load, b))
    return bin_of, lane_of


def _prep(cfg: Cfg, x, edge_index, edge_attr, W1, b1, g1, beta1, m1, v1,
          W2, b2, g2, beta2, m2, v2, W3, b3):
    bf16 = ml_dtypes.bfloat16
    n, e = cfg.n, cfg.e
    src = np.asarray(edge_index[0], dtype=np.int64)
    dst = np.asarray(edge_index[1], dtype=np.int64)
    ew = np.asarray(edge_attr, dtype=np.float32).mean(axis=1)

    indeg = np.bincount(dst, minlength=n)
    bin_of, lane_of = _balance_nodes(indeg, cfg)
    core_of = bin_of // cfg.tiles
    tile_of = bin_of % cfg.tiles

    # edge count per bin -> per-core tile ordering (descending count)
    ecnt = np.bincount(bin_of[dst], minlength=cfg.ncores * cfg.tiles)
    ecnt = ecnt.reshape(cfg.ncores, cfg.tiles)
    pos_of_tile = np.empty((cfg.ncores, cfg.tiles), dtype=np.int64)
    for c in range(cfg.ncores):
        order_t = np.argsort(-ecnt[c], kind="stable")
        pos_of_tile[c, order_t] = np.arange(cfg.tiles)

    # global row layout (chunk-major for chunked AllGather)
    pos_of_node = pos_of_tile[core_of, tile_of]
    chunk_of_pos_arr = np.array([_chunk_of_pos(p) for p in range(cfg.tiles)])
    chunk_of_node = chunk_of_pos_arr[pos_of_node]
    cbase = np.array(CHUNK_BASE[:NCHUNKS])
    ctiles = np.array(CHUNK_TILES)
    cp0 = np.array(CHUNK_P0)
    row_of_node = (cbase[chunk_of_node]
                   + core_of * P * ctiles[chunk_of_node]
                   + (pos_of_node - cp0[chunk_of_node]) * P
                   + lane_of)
    # local slice row (position-major within core)
    lrow_of_node = pos_of_node * P + lane_of

    # ---- edge grouping: key = (core, pos, is_hi) ----
    e_core = core_of[dst]
    e_pos = pos_of_node[dst]
    e_srow = row_of_node[src]
    e_hi = (e_srow >= HALF_ROWS).astype(np.int64)
    e_dlocal = lane_of[dst]

    key = (e_core * cfg.tiles + e_pos) * 2 + e_hi
    order = np.argsort(key, kind="stable")
    key_s = key[order]
    nkeys = cfg.ncores * cfg.tiles * 2
    counts_g = np.bincount(key_s, minlength=nkeys)
    gstart = np.zeros(nkeys, dtype=np.int64)
    gstart[1:] = np.cumsum(counts_g)[:-1]
    rank = np.arange(e, dtype=np.int64) - gstart[key_s]

    cg = counts_g.reshape(cfg.ncores, cfg.tiles, 2)
    # per-position chunk counts (max over cores)
    ct_lo = np.maximum(1, np.ceil(cg[:, :, 0].max(axis=0) / P).astype(np.int64))
    ct_hi = np.maximum(1, np.ceil(cg[:, :, 1].max(axis=0) / P).astype(np.int64))
    cb = np.zeros(cfg.tiles + 1, dtype=np.int64)
    cb[1:] = np.cumsum(ct_lo + ct_hi)
    CT = int(cb[-1])

    # per-edge placement in the chunk grid
    se_core = e_core[order]
    se_pos = e_pos[order]
    se_hi = e_hi[order]
    se_chunk = cb[se_pos] + se_hi * ct_lo[se_pos] + rank // P
    se_lane = rank % P
    se_srow = e_srow[order] - se_hi * HALF_ROWS
    se_dlocal = e_dlocal[order]
    se_w = ew[order].astype(np.float32)

    # dslot / weight tables [core, 128, CT]
    dslot = np.zeros((cfg.ncores, P, CT), dtype=np.float32)
    w12 = np.zeros((cfg.ncores, P, CT), dtype=np.float32)
    w3 = np.zeros((cfg.ncores, P, CT), dtype=np.float32)
    dslot[se_core, se_lane, se_chunk] = se_dlocal
    w12[se_core, se_lane, se_chunk] = se_w
    w3[se_core, se_lane, se_chunk] = 1.0

    # gather index table [core, 128, CT*8] int16, pad = -1
    idx_lin = np.full((cfg.ncores, CT * P), -1 if NEG_IDX else 0, dtype=np.int64)
    idx_lin[se_core, se_chunk * P + se_lane] = se_srow
    # ensure pads are trailing within each (pos, half) call range: edges fill
    # rank-contiguously from each group start, so pads are trailing. ✓
    idx16 = idx_lin.astype(np.int16).reshape(cfg.ncores, CT * 8, 16)
    idx_sb = np.zeros((cfg.ncores, P, CT * 8), dtype=np.int16)
    idx_sb[:, :16, :] = idx16.transpose(0, 2, 1)
    idx_sb[:, 16:, :] = np.tile(idx_sb[:, :16, :], (1, 7, 1))

    # x table in row layout [total_slots, CIN] bf16
    xf = np.asarray(x, dtype=np.float32)
    xfull = np.zeros((cfg.total_slots, CIN), dtype=np.float32)
    xfull[row_of_node] = xf
    xfull = xfull.astype(bf16)

    # weights / epilogue params (BN scale folded into W; bias via ones-row)
    eps = 1e-5
    s1 = (np.asarray(g1) / np.sqrt(np.asarray(v1) + eps)).astype(np.float32)
    t1 = (np.asarray(beta1) + (np.asarray(b1) - np.asarray(m1)) * s1).astype(np.float32)
    s2 = (np.asarray(g2) / np.sqrt(np.asarray(v2) + eps)).astype(np.float32)
    t2 = (np.asarray(beta2) + (np.asarray(b2) - np.asarray(m2)) * s2).astype(np.float32)

    w1p = (s1[:, None] * np.asarray(W1, np.float32)).T.astype(bf16)  # [128,256]
    w2p = (s2[:, None] * np.asarray(W2, np.float32)).T.reshape(2, P, HIDDEN).astype(bf16)
    w3p = np.asarray(W3, np.float32).T.reshape(2, P, HIDDEN).astype(bf16)

    def btile(v):
        t = np.zeros((P, HIDDEN), dtype=np.float32)
        t[0, :] = v
        return t.astype(bf16)

    ones_row = np.zeros((P, P), dtype=np.float32)
    ones_row[0, :] = 1.0
    iota = np.broadcast_to(np.arange(P, dtype=np.float32), (P, P)).copy()

    in_maps = []
    for c in range(cfg.ncores):
        in_maps.append({
            "xfull": np.ascontiguousarray(xfull),
            "idx": np.ascontiguousarray(idx_sb[c]),
            "dslot": np.ascontiguousarray(dslot[c]),
            "w12": np.ascontiguousarray(w12[c]),
            "w3": np.ascontiguousarray(w3[c]),
            "w1p": w1p, "w2p": w2p, "w3p": w3p,
            "b1t": btile(t1), "b2t": btile(t2),
            "b3t": btile(np.asarray(b3, np.float32)),
            "ones": ones_row.astype(bf16),
            "iota": iota.astype(bf16),
        })
    meta = dict(ct_lo=ct_lo.tolist(), ct_hi=ct_hi.tolist(), cb=cb.tolist(),
                CT=CT, lrow_of_node=lrow_of_node, core_of=core_of)
    return in_maps, meta


# ---------------------------------------------------------------------------
# Bass program
# ---------------------------------------------------------------------------

def _build(cfg: Cfg, meta):
    import concourse.mybir as mybir
    import concourse.tile as tile
    from concourse import bacc

    ct_lo, ct_hi, cb, CT = meta["ct_lo"], meta["ct_hi"], meta["cb"], meta["CT"]
    CTLO = max(ct_lo)
    CTHI = max(ct_hi)
    T = cfg.tiles
    SPC = cfg.slots_per_core
    TOT = cfg.total_slots
    DT = mybir.dt
    AOP = mybir.AluOpType
    ACT = mybir.ActivationFunctionType
    nc = bacc.Bacc("TRN2", target_bir_lowering=False, debug=False,
                   num_devices=cfg.ncores, num_swdge_queues=4)

    xfull_d = nc.declare_dram_parameter("xfull", [TOT, CIN], DT.bfloat16, isOutput=False)
    idx_d = nc.declare_dram_parameter("idx", [P, CT * 8], DT.int16, isOutput=False)
    dslot_d = nc.declare_dram_parameter("dslot", [P, CT], DT.float32, isOutput=False)
    w12_d = nc.declare_dram_parameter("w12", [P, CT], DT.float32, isOutput=False)
    w3_d = nc.declare_dram_parameter("w3", [P, CT], DT.float32, isOutput=False)
    w1p_d = nc.declare_dram_parameter("w1p", [CIN, HIDDEN], DT.bfloat16, isOutput=False)
    w2p_d = nc.declare_dram_parameter("w2p", [2, P, HIDDEN], DT.bfloat16, isOutput=False)
    w3p_d = nc.declare_dram_parameter("w3p", [2, P, HIDDEN], DT.bfloat16, isOutput=False)
    b1t_d = nc.declare_dram_parameter("b1t", [P, HIDDEN], DT.bfloat16, isOutput=False)
    b2t_d = nc.declare_dram_parameter("b2t", [P, HIDDEN], DT.bfloat16, isOutput=False)
    b3t_d = nc.declare_dram_parameter("b3t", [P, HIDDEN], DT.bfloat16, isOutput=False)
    ones_d = nc.declare_dram_parameter("ones", [P, P], DT.bfloat16, isOutput=False)
    iota_d = nc.declare_dram_parameter("iota", [P, P], DT.bfloat16, isOutput=False)
    out_d = nc.declare_dram_parameter("out", [SPC, HIDDEN], DT.float32, isOutput=True)

    zslice = [nc.dram_tensor(f"zslice{l}", [SPC, HIDDEN], DT.bfloat16)
              for l in range(2)]
    zfull = [nc.dram_tensor(f"zfull{l}", [TOT, HIDDEN], DT.bfloat16,
                            addr_space="Shared")
             for l in range(2)]
    groups = [list(range(cfg.ncores))]

    with tile.TileContext(nc) as tc:
        with (
            tc.tile_pool(name="const", bufs=1) as const_pool,
            tc.tile_pool(name="mpool", bufs=10) as m_pool,
            tc.tile_pool(name="g1pool", bufs=3) as g1_pool,
            tc.tile_pool(name="gpool", bufs=3) as g_pool,
            tc.tile_pool(name="epool", bufs=3) as e_pool,
            tc.tile_pool(name="zpool", bufs=3) as z_pool,
            tc.tile_pool(name="agg_ps", bufs=2, space="PSUM") as agg_psum,
            tc.tile_pool(name="agg1_ps", bufs=2, space="PSUM") as agg1_psum,
            tc.tile_pool(name="z_ps", bufs=2, space="PSUM") as z_psum,
            tc.tile_pool(name="tr_ps", bufs=2, space="PSUM") as tr_psum,
        ):
            # persistent tiles
            idx_sb = const_pool.tile([P, CT * 8], DT.int16)
            nc.sync.dma_start(idx_sb[:], idx_d[:])
            dslot_sb = const_pool.tile([P, CT], DT.float32)
            nc.sync.dma_start(dslot_sb[:], dslot_d[:])
            w12_sb = const_pool.tile([P, CT], DT.float32)
            nc.sync.dma_start(w12_sb[:], w12_d[:])
            w3_sb = const_pool.tile([P, CT], DT.float32)
            nc.sync.dma_start(w3_sb[:], w3_d[:])
            w1p_sb = const_pool.tile([CIN, HIDDEN], DT.bfloat16)
            nc.sync.dma_start(w1p_sb[:], w1p_d[:])
            w2p_sb = const_pool.tile([P, 2, HIDDEN], DT.bfloat16)
            nc.sync.dma_start(w2p_sb[:], w2p_d[:].rearrange("h p n -> p h n"))
            w3p_sb = const_pool.tile([P, 2, HIDDEN], DT.bfloat16)
            nc.sync.dma_start(w3p_sb[:], w3p_d[:].rearrange("h p n -> p h n"))
            b_sb = []
            for i, d in enumerate((b1t_d, b2t_d, b3t_d)):
                t_ = const_pool.tile([P, HIDDEN], DT.bfloat16, tag=f"b{i}")
                nc.sync.dma_start(t_[:], d[:])
                b_sb.append(t_)
            ones_sb = const_pool.tile([P, P], DT.bfloat16)
            nc.sync.dma_start(ones_sb[:], ones_d[:])
            iota_sb = const_pool.tile([P, P], DT.bfloat16)
            nc.sync.dma_start(iota_sb[:], iota_d[:])
            if not USE_VEC_TRANSPOSE:
                from concourse.masks import make_identity
                ident = const_pool.tile([P, P], DT.bfloat16, tag="ident")
                make_identity(nc, ident[:])
            zero_sb = const_pool.tile([P, HIDDEN], DT.float32, tag="zero")
            nc.vector.memset(zero_sb[:], 0.0)

            if NEG_IDX:
                # prime gather-pool buffers (skipped rows must stay finite)
                for _ in range(3):
                    for tg, pl, w_ in (("glo1", g1_pool, CTLO), ("ghi1", g1_pool, CTHI),
                                       ("glo", g_pool, CTLO), ("ghi", g_pool, CTHI)):
                        el = CIN if tg.endswith("1") else HIDDEN
                        t_ = pl.tile([P, w_, el], DT.bfloat16, tag=tg)
                        nc.vector.memset(t_[:], 0.0)

            for l in range(3):
                elem = CIN if l == 0 else HIDDEN
                nh = 1 if l == 0 else 2
                pool = g1_pool if l == 0 else g_pool
                gtag = ("glo1", "ghi1") if l == 0 else ("glo", "ghi")
                table = xfull_d if l == 0 else zfull[l - 1]
                wcol_sb = w12_sb if l < 2 else w3_sb
                psum_pool = agg1_psum if l == 0 else agg_psum
                wp_sb = (w1p_sb, w2p_sb, w3p_sb)[l]

                for p in range(T):
                    ctl, cth = ct_lo[p], ct_hi[p]
                    cbp = cb[p]
                    glo = pool.tile([P, CTLO, elem], DT.bfloat16, tag=gtag[0])
                    nc.gpsimd.dma_gather(
                        glo[:, :ctl, :], table[0:HALF_ROWS, :],
                        idx_sb[:, cbp * 8: cbp * 8 + ctl * 8],
                        ctl * P, ctl * P, elem, single_packet=False,
                        queue_num=(2 * p) % 4)
                    ghi = pool.tile([P, CTHI, elem], DT.bfloat16, tag=gtag[1])
                    nc.gpsimd.dma_gather(
                        ghi[:, :cth, :], table[HALF_ROWS:, :],
                        idx_sb[:, cbp * 8 + ctl * 8: cbp * 8 + (ctl + cth) * 8],
                        cth * P, cth * P, elem, single_packet=False,
                        queue_num=(2 * p + 1) % 4)

                    aps = psum_pool.tile([P, elem], DT.float32)
                    ct = ctl + cth
                    for k in range(ct):
                        g = glo[:, k, :] if k < ctl else ghi[:, k - ctl, :]
                        m = m_pool.tile([P, P], DT.bfloat16)
                        col = cbp + k
                        nc.vector.tensor_scalar(
                            m[:], iota_sb[:],
                            dslot_sb[:, col:col + 1],
                            wcol_sb[:, col:col + 1],
                            op0=AOP.is_equal, op1=AOP.mult)
                        nc.tensor.matmul(aps[:], m[:], g,
                                         start=(k == 0), stop=(k == ct - 1))

                    # agg psum -> sbuf bf16, then transpose
                    agg_sb = e_pool.tile([P, elem], DT.bfloat16, tag="agg")
                    if USE_SCALAR_ENG:
                        nc.scalar.copy(agg_sb[:], aps[:])
                    else:
                        nc.vector.tensor_copy(agg_sb[:], aps[:])
                    aggT = e_pool.tile([P, nh, P], DT.bfloat16, tag="aggT")
                    for h in range(nh):
                        if USE_VEC_TRANSPOSE:
                            nc.vector.transpose(
                                aggT[:, h, :], agg_sb[:, h * P:(h + 1) * P])
                        else:
                            tp = tr_psum.tile([P, P], DT.bfloat16)
                            nc.tensor.transpose(
                                tp[:], agg_sb[:, h * P:(h + 1) * P], ident[:])
                            nc.vector.tensor_copy(aggT[:, h, :], tp[:])

                    # dense layer + bias (ones-row chunk)
                    zps = z_psum.tile([P, HIDDEN], DT.float32, tag="zps")
                    for h in range(nh):
                        wslice = wp_sb[:] if l == 0 else wp_sb[:, h, :]
                        nc.tensor.matmul(zps[:], aggT[:, h, :], wslice,
                                         start=(h == 0), stop=False)
                    nc.tensor.matmul(zps[:], ones_sb[:], b_sb[l][:],
                                     start=False, stop=True)

                    if l < 2:
                        hsb = z_pool.tile([P, HIDDEN], DT.bfloat16, tag="h")
                        if USE_SCALAR_ENG:
                            nc.scalar.activation(hsb[:], zps[:], ACT.Relu)
                        else:
                            nc.vector.tensor_tensor(
                                out=hsb[:], in0=zps[:], in1=zero_sb[:],
                                op=AOP.max)
                        nc.sync.dma_start(
                            zslice[l][p * P:(p + 1) * P, :], hsb[:])
                        # fire AllGather chunk when its last tile completes
                        cidx = _chunk_of_pos(p)
                        if p == CHUNK_P0[cidx] + CHUNK_TILES[cidx] - 1:
                            r0 = CHUNK_P0[cidx] * P
                            r1 = r0 + CHUNK_TILES[cidx] * P
                            g0 = CHUNK_BASE[cidx]
                            g1_ = g0 + 8 * CHUNK_TILES[cidx] * P
                            nc.gpsimd.collective_compute(
                                "AllGather", AOP.bypass,
                                replica_groups=groups,
                                ins=[zslice[l][r0:r1, :]],
                                outs=[zfull[l][g0:g1_, :]])
                    else:
                        osb = z_pool.tile([P, HIDDEN], DT.float32, tag="o")
                        if USE_SCALAR_ENG:
                            nc.scalar.copy(osb[:], zps[:])
                        else:
                            nc.vector.tensor_copy(osb[:], zps[:])
                        nc.sync.dma_start(
                            out_d[p * P:(p + 1) * P, :], osb[:])
    nc.compile()
    return nc


# ---------------------------------------------------------------------------
# Entry point
# ---------------------------------------------------------------------------

LAST_RESULTS = None


def _run(cfg: Cfg, inputs: dict, trace: bool = False,
         trace_cores=None) -> np.ndarray:
    global LAST_RESULTS
    from concourse.bass_utils import run_bass_kernel_spmd

    in_maps, meta = _prep(cfg, **inputs)
    nc = _build(cfg, meta)
    kr = run_bass_kernel_spmd(nc, in_maps, list(range(cfg.ncores)), trace=trace,
                              trace_cores=trace_cores)
    LAST_RESULTS = kr
    res = kr.results
    out = np.empty((cfg.n, HIDDEN), dtype=np.float32)
    lrow = meta["lrow_of_node"]
    core = meta["core_of"]
    full = np.stack([res[c]["out"] for c in range(cfg.ncores)], axis=0)
    out[np.arange(cfg.n)] = full[core, lrow]
    return out


def kernel(**inputs) -> np.ndarray:
    return _run(CFG, inputs)


# revision 9
# speedup vs baseline: 2.5550x; 2.5550x over previous
"""GCN encoder (3-layer, PyG GCNConv normalize=False + BN eval + ReLU) on 8 trn2 cores.

v2 strategy (node/dst-sharded, graph-parallel, reassociated):
  - agg_l = A @ (h W^T) is computed as (A @ h) W^T: per dst tile, gather h rows
    for the tile's edges (dma_gather, int16 idx, lo/hi table split), aggregate
    with 128-edge one-hot matmul chains (one-hot scatter matrices generated
    ON-CHIP on DVE via iota/is_equal/mult from compact dslot/weight vectors),
    then apply the dense layer per tile (DVE transpose + 2 matmuls) with the
    BN scale folded into W and the bias added via a ones-row matmul chunk.
  - Layer 1 gathers directly from a replicated x table (no z1 AllGather).
  - h1/h2 tables are AllGathered in 4 chunks, overlapped with tile compute.
  - Padding lanes use negative gather indices (skipped by the DMA) and
    zero weights in the on-chip scatter matrices.
"""

import math
from dataclasses import dataclass

import ml_dtypes
import numpy as np

P = 128
HIDDEN = 256
CIN = 128

# feature flags (conservative=False uses baseline-proven idioms)
NEG_IDX = False        # pad gather idx with -1 (DMA skips) vs 0 (gathers row 0)
USE_SCALAR_ENG = False # psum->sbuf copies + relu on scalar engine vs DVE
USE_VEC_TRANSPOSE = False  # DVE transpose vs PE transpose
NCHUNKS = 1            # AllGather chunks per layer boundary


@dataclass
class Cfg:
    n: int = 50000
    e: int = 1600000
    ncores: int = 8
    tiles: int = 49

    @property
    def slots_per_core(self) -> int:
        return self.tiles * P

    @property
    def total_slots(self) -> int:
        return self.ncores * self.slots_per_core


CFG = Cfg()

# tile positions per AG chunk (sums to 49)
if NCHUNKS == 4:
    CHUNK_TILES = [13, 12, 12, 12]
elif NCHUNKS == 2:
    CHUNK_TILES = [25, 24]
else:
    CHUNK_TILES = [49]
CHUNK_P0 = [0]
for _t in CHUNK_TILES[:-1]:
    CHUNK_P0.append(CHUNK_P0[-1] + _t)
# global row base of each chunk region in the full table
CHUNK_BASE = [0]
for _t in CHUNK_TILES:
    CHUNK_BASE.append(CHUNK_BASE[-1] + 8 * P * _t)
# lo/hi gather-table boundary; must be < 32768 and at a chunk/core boundary
if NCHUNKS == 1:
    HALF_ROWS = 4 * P * 49  # core 0-3 vs 4-7 (rank-major layout), 25088
else:
    HALF_ROWS = CHUNK_BASE[2]


def _chunk_of_pos(p: int) -> int:
    for c in range(NCHUNKS - 1, -1, -1):
        if p >= CHUNK_P0[c]:
            return c
    return 0


# ---------------------------------------------------------------------------
# Host-side preprocessing
# ---------------------------------------------------------------------------

def _balance_nodes(indeg: np.ndarray, cfg: Cfg) -> tuple[np.ndarray, np.ndarray]:
    """Assign nodes to (bin = core*tiles + tile, lane) balancing per-bin indeg.
    Returns bin_of[n], lane_of[n]."""
    import heapq

    nbins = cfg.ncores * cfg.tiles
    order = np.argsort(-indeg, kind="stable")
    heap = [(0, b) for b in range(nbins)]
    heapq.heapify(heap)
    counts = np.zeros(nbins, dtype=np.int64)
    bin_of = np.empty(cfg.n, dtype=np.int64)
    lane_of = np.empty(cfg.n, dtype=np.int64)
    for v in order:
        load, b = heapq.heappop(heap)
        bin_of[v] = b
        lane_of[v] = counts[b]
        counts[b] += 1
        load += int(indeg[v])
        if counts[b] < P:
            heapq.heappush(heap, (load, b))
    return bin_of, lane_of


def _prep(cfg: Cfg, x, edge_index, edge_attr, W1, b1, g1, beta1, m1, v1,
          W2, b2, g2, beta2, m2, v2, W3, b3):
    bf16 = ml_dtypes.bfloat16
    n, e = cfg.n, cfg.e
    src = np.asarray(edge_index[0], dtype=np.int64)
    dst = np.asarray(edge_index[1], dtype=np.int64)
    ew = np.asarray(edge_attr, dtype=np.float32).mean(axis=1)

    indeg = np.bincount(dst, minlength=n)
    bin_of, lane_of = _balance_nodes(indeg, cfg)
    core_of = bin_of // cfg.tiles
    tile_of = bin_of % cfg.tiles

    # edge count per bin -> per-core tile ordering (descending count)
    ecnt = np.bincount(bin_of[dst], minlength=cfg.ncores * cfg.tiles)
    ecnt = ecnt.reshape(cfg.ncores, cfg.tiles)
    pos_of_tile = np.empty((cfg.ncores, cfg.tiles), dtype=np.int64)
    for c in range(cfg.ncores):
        order_t = np.argsort(-ecnt[c], kind="stable")
        pos_of_tile[c, order_t] = np.arange(cfg.tiles)

    # global row layout (chunk-major for chunked AllGather)
    pos_of_node = pos_of_tile[core_of, tile_of]
    chunk_of_pos_arr = np.array([_chunk_of_pos(p) for p in range(cfg.tiles)])
    chunk_of_node = chunk_of_pos_arr[pos_of_node]
    cbase = np.array(CHUNK_BASE[:NCHUNKS])
    ctiles = np.array(CHUNK_TILES)
    cp0 = np.array(CHUNK_P0)
    row_of_node = (cbase[chunk_of_node]
                   + core_of * P * ctiles[chunk_of_node]
                   + (pos_of_node - cp0[chunk_of_node]) * P
                   + lane_of)
    # local slice row (position-major within core)
    lrow_of_node = pos_of_node * P + lane_of

    # ---- edge grouping: key = (core, pos, is_hi) ----
    e_core = core_of[dst]
    e_pos = pos_of_node[dst]
    e_srow = row_of_node[src]
    e_hi = (e_srow >= HALF_ROWS).astype(np.int64)
    e_dlocal = lane_of[dst]

    key = (e_core * cfg.tiles + e_pos) * 2 + e_hi
    order = np.argsort(key, kind="stable")
    key_s = key[order]
    nkeys = cfg.ncores * cfg.tiles * 2
    counts_g = np.bincount(key_s, minlength=nkeys)
    gstart = np.zeros(nkeys, dtype=np.int64)
    gstart[1:] = np.cumsum(counts_g)[:-1]
    rank = np.arange(e, dtype=np.int64) - gstart[key_s]

    cg = counts_g.reshape(cfg.ncores, cfg.tiles, 2)
    # per-position chunk counts (max over cores)
    ct_lo = np.maximum(1, np.ceil(cg[:, :, 0].max(axis=0) / P).astype(np.int64))
    ct_hi = np.maximum(1, np.ceil(cg[:, :, 1].max(axis=0) / P).astype(np.int64))
    cb = np.zeros(cfg.tiles + 1, dtype=np.int64)
    cb[1:] = np.cumsum(ct_lo + ct_hi)
    CT = int(cb[-1])

    # per-edge placement in the chunk grid
    se_core = e_core[order]
    se_pos = e_pos[order]
    se_hi = e_hi[order]
    se_chunk = cb[se_pos] + se_hi * ct_lo[se_pos] + rank // P
    se_lane = rank % P
    se_srow = e_srow[order] - se_hi * HALF_ROWS
    se_dlocal = e_dlocal[order]
    se_w = ew[order].astype(np.float32)

    # dslot / weight tables [core, 128, CT]
    dslot = np.zeros((cfg.ncores, P, CT), dtype=np.float32)
    w12 = np.zeros((cfg.ncores, P, CT), dtype=np.float32)
    w3 = np.zeros((cfg.ncores, P, CT), dtype=np.float32)
    dslot[se_core, se_lane, se_chunk] = se_dlocal
    w12[se_core, se_lane, se_chunk] = se_w
    w3[se_core, se_lane, se_chunk] = 1.0

    # gather index table [core, 128, CT*8] int16, pad = -1
    idx_lin = np.full((cfg.ncores, CT * P), -1 if NEG_IDX else 0, dtype=np.int64)
    idx_lin[se_core, se_chunk * P + se_lane] = se_srow
    # ensure pads are trailing within each (pos, half) call range: edges fill
    # rank-contiguously from each group start, so pads are trailing. ✓
    idx16 = idx_lin.astype(np.int16).reshape(cfg.ncores, CT * 8, 16)
    idx_sb = np.zeros((cfg.ncores, P, CT * 8), dtype=np.int16)
    idx_sb[:, :16, :] = idx16.transpose(0, 2, 1)
    idx_sb[:, 16:, :] = np.tile(idx_sb[:, :16, :], (1, 7, 1))

    # x table in row layout [total_slots, CIN] bf16
    xf = np.asarray(x, dtype=np.float32)
    xfull = np.zeros((cfg.total_slots, CIN), dtype=np.float32)
    xfull[row_of_node] = xf
    xfull = xfull.astype(bf16)

    # weights / epilogue params (BN scale folded into W; bias via ones-row)
    eps = 1e-5
    s1 = (np.asarray(g1) / np.sqrt(np.asarray(v1) + eps)).astype(np.float32)
    t1 = (np.asarray(beta1) + (np.asarray(b1) - np.asarray(m1)) * s1).astype(np.float32)
    s2 = (np.asarray(g2) / np.sqrt(np.asarray(v2) + eps)).astype(np.float32)
    t2 = (np.asarray(beta2) + (np.asarray(b2) - np.asarray(m2)) * s2).astype(np.float32)

    w1p = (s1[:, None] * np.asarray(W1, np.float32)).T.astype(bf16)  # [128,256]
    w2p = (s2[:, None] * np.asarray(W2, np.float32)).T.reshape(2, P, HIDDEN).astype(bf16)
    w3p = np.asarray(W3, np.float32).T.reshape(2, P, HIDDEN).astype(bf16)

    def btile(v):
        t = np.zeros((P, HIDDEN), dtype=np.float32)
        t[0, :] = v
        return t.astype(bf16)

    ones_row = np.zeros((P, P), dtype=np.float32)
    ones_row[0, :] = 1.0
    iota = np.broadcast_to(np.arange(P, dtype=np.float32), (P, P)).copy()

    in_maps = []
    for c in range(cfg.ncores):
        in_maps.append({
            "xfull": np.ascontiguousarray(xfull),
            "idx": np.ascontiguousarray(idx_sb[c]),
            "dslot": np.ascontiguousarray(dslot[c].astype(bf16)),
            "w12": np.ascontiguousarray(w12[c].astype(bf16)),
            "w3": np.ascontiguousarray(w3[c].astype(bf16)),
            "w1p": w1p, "w2p": w2p, "w3p": w3p,
            "b1t": btile(t1), "b2t": btile(t2),
            "b3t": btile(np.asarray(b3, np.float32)),
            "ones": ones_row.astype(bf16),
            "iota": iota.astype(bf16),
        })
    meta = dict(ct_lo=ct_lo.tolist(), ct_hi=ct_hi.tolist(), cb=cb.tolist(),
                CT=CT, lrow_of_node=lrow_of_node, core_of=core_of)
    return in_maps, meta


# ---------------------------------------------------------------------------
# Bass program
# ---------------------------------------------------------------------------

def _build(cfg: Cfg, meta):
    import concourse.mybir as mybir
    import concourse.tile as tile
    from concourse import bacc

    ct_lo, ct_hi, cb, CT = meta["ct_lo"], meta["ct_hi"], meta["cb"], meta["CT"]
    CTLO = max(ct_lo)
    CTHI = max(ct_hi)
    CTMAX = max(l + h for l, h in zip(ct_lo, ct_hi))
    T = cfg.tiles
    SPC = cfg.slots_per_core
    TOT = cfg.total_slots
    DT = mybir.dt
    AOP = mybir.AluOpType
    ACT = mybir.ActivationFunctionType
    nc = bacc.Bacc("TRN2", target_bir_lowering=False, debug=False,
                   num_devices=cfg.ncores, num_swdge_queues=4)

    xfull_d = nc.declare_dram_parameter("xfull", [TOT, CIN], DT.bfloat16, isOutput=False)
    idx_d = nc.declare_dram_parameter("idx", [P, CT * 8], DT.int16, isOutput=False)
    dslot_d = nc.declare_dram_parameter("dslot", [P, CT], DT.bfloat16, isOutput=False)
    w12_d = nc.declare_dram_parameter("w12", [P, CT], DT.bfloat16, isOutput=False)
    w3_d = nc.declare_dram_parameter("w3", [P, CT], DT.bfloat16, isOutput=False)
    w1p_d = nc.declare_dram_parameter("w1p", [CIN, HIDDEN], DT.bfloat16, isOutput=False)
    w2p_d = nc.declare_dram_parameter("w2p", [2, P, HIDDEN], DT.bfloat16, isOutput=False)
    w3p_d = nc.declare_dram_parameter("w3p", [2, P, HIDDEN], DT.bfloat16, isOutput=False)
    b1t_d = nc.declare_dram_parameter("b1t", [P, HIDDEN], DT.bfloat16, isOutput=False)
    b2t_d = nc.declare_dram_parameter("b2t", [P, HIDDEN], DT.bfloat16, isOutput=False)
    b3t_d = nc.declare_dram_parameter("b3t", [P, HIDDEN], DT.bfloat16, isOutput=False)
    ones_d = nc.declare_dram_parameter("ones", [P, P], DT.bfloat16, isOutput=False)
    iota_d = nc.declare_dram_parameter("iota", [P, P], DT.bfloat16, isOutput=False)
    out_d = nc.declare_dram_parameter("out", [SPC, HIDDEN], DT.float32, isOutput=True)

    zslice = [nc.dram_tensor(f"zslice{l}", [SPC, HIDDEN], DT.bfloat16)
              for l in range(2)]
    zfull = [nc.dram_tensor(f"zfull{l}", [TOT, HIDDEN], DT.bfloat16,
                            addr_space="Shared")
             for l in range(2)]
    groups = [list(range(cfg.ncores))]

    with tile.TileContext(nc) as tc:
        with (
            tc.tile_pool(name="const", bufs=1) as const_pool,
            tc.tile_pool(name="mpool", bufs=3) as m_pool,
            tc.tile_pool(name="g1pool", bufs=3) as g1_pool,
            tc.tile_pool(name="gpool", bufs=3) as g_pool,
            tc.tile_pool(name="epool", bufs=3) as e_pool,
            tc.tile_pool(name="zpool", bufs=3) as z_pool,
            tc.tile_pool(name="agg_ps", bufs=2, space="PSUM") as agg_psum,
            tc.tile_pool(name="agg1_ps", bufs=2, space="PSUM") as agg1_psum,
            tc.tile_pool(name="z_ps", bufs=2, space="PSUM") as z_psum,
            tc.tile_pool(name="tr_ps", bufs=2, space="PSUM") as tr_psum,
        ):
            # persistent tiles
            idx_sb = const_pool.tile([P, CT * 8], DT.int16)
            nc.sync.dma_start(idx_sb[:], idx_d[:])
            dslot_sb = const_pool.tile([P, CT], DT.bfloat16)
            nc.sync.dma_start(dslot_sb[:], dslot_d[:])
            w12_sb = const_pool.tile([P, CT], DT.bfloat16)
            nc.sync.dma_start(w12_sb[:], w12_d[:])
            w3_sb = const_pool.tile([P, CT], DT.bfloat16)
            nc.sync.dma_start(w3_sb[:], w3_d[:])
            w1p_sb = const_pool.tile([CIN, HIDDEN], DT.bfloat16)
            nc.sync.dma_start(w1p_sb[:], w1p_d[:])
            w2p_sb = const_pool.tile([P, 2, HIDDEN], DT.bfloat16)
            nc.sync.dma_start(w2p_sb[:], w2p_d[:].rearrange("h p n -> p h n"))
            w3p_sb = const_pool.tile([P, 2, HIDDEN], DT.bfloat16)
            nc.sync.dma_start(w3p_sb[:], w3p_d[:].rearrange("h p n -> p h n"))
            b_sb = []
            for i, d in enumerate((b1t_d, b2t_d, b3t_d)):
                t_ = const_pool.tile([P, HIDDEN], DT.bfloat16, tag=f"b{i}")
                nc.sync.dma_start(t_[:], d[:])
                b_sb.append(t_)
            ones_sb = const_pool.tile([P, P], DT.bfloat16)
            nc.sync.dma_start(ones_sb[:], ones_d[:])
            iota_sb = const_pool.tile([P, P], DT.bfloat16)
            nc.sync.dma_start(iota_sb[:], iota_d[:])
            if not USE_VEC_TRANSPOSE:
                from concourse.masks import make_identity
                ident = const_pool.tile([P, P], DT.bfloat16, tag="ident")
                make_identity(nc, ident[:])
            zero_sb = const_pool.tile([P, HIDDEN], DT.float32, tag="zero")
            nc.vector.memset(zero_sb[:], 0.0)

            if NEG_IDX:
                # prime gather-pool buffers (skipped rows must stay finite)
                for _ in range(3):
                    for tg, pl, w_ in (("glo1", g1_pool, CTLO), ("ghi1", g1_pool, CTHI),
                                       ("glo", g_pool, CTLO), ("ghi", g_pool, CTHI)):
                        el = CIN if tg.endswith("1") else HIDDEN
                        t_ = pl.tile([P, w_, el], DT.bfloat16, tag=tg)
                        nc.vector.memset(t_[:], 0.0)

            for l in range(3):
                elem = CIN if l == 0 else HIDDEN
                nh = 1 if l == 0 else 2
                pool = g1_pool if l == 0 else g_pool
                gtag = ("glo1", "ghi1") if l == 0 else ("glo", "ghi")
                table = xfull_d if l == 0 else zfull[l - 1]
                wcol_sb = w12_sb if l < 2 else w3_sb
                psum_pool = agg1_psum if l == 0 else agg_psum
                wp_sb = (w1p_sb, w2p_sb, w3p_sb)[l]

                for p in range(T):
                    ctl, cth = ct_lo[p], ct_hi[p]
                    cbp = cb[p]
                    glo = pool.tile([P, CTLO, elem], DT.bfloat16, tag=gtag[0])
                    nc.gpsimd.dma_gather(
                        glo[:, :ctl, :], table[0:HALF_ROWS, :],
                        idx_sb[:, cbp * 8: cbp * 8 + ctl * 8],
                        ctl * P, ctl * P, elem, single_packet=False,
                        queue_num=(2 * p) % 4)
                    ghi = pool.tile([P, CTHI, elem], DT.bfloat16, tag=gtag[1])
                    nc.gpsimd.dma_gather(
                        ghi[:, :cth, :], table[HALF_ROWS:, :],
                        idx_sb[:, cbp * 8 + ctl * 8: cbp * 8 + (ctl + cth) * 8],
                        cth * P, cth * P, elem, single_packet=False,
                        queue_num=(2 * p + 1) % 4)

                    aps = psum_pool.tile([P, elem], DT.float32)
                    ct = ctl + cth
                    # batched one-hot scatter-matrix generation: 2 DVE ops
                    # M[lane, k, d] = (iota[d] == dslot[lane, cbp+k]) * w[lane, cbp+k]
                    mt = m_pool.tile([P, CTMAX, P], DT.bfloat16)
                    iota_b = iota_sb[:].rearrange("p (u d) -> p u d", u=1).to_broadcast((P, ct, P))
                    dsl_b = dslot_sb[:, cbp:cbp + ct].rearrange("p (c u) -> p c u", u=1).to_broadcast((P, ct, P))
                    w_b = wcol_sb[:, cbp:cbp + ct].rearrange("p (c u) -> p c u", u=1).to_broadcast((P, ct, P))
                    nc.vector.tensor_tensor(
                        out=mt[:, :ct, :], in0=iota_b, in1=dsl_b, op=AOP.is_equal)
                    nc.vector.tensor_tensor(
                        out=mt[:, :ct, :], in0=mt[:, :ct, :], in1=w_b, op=AOP.mult)
                    for k in range(ct):
                        g = glo[:, k, :] if k < ctl else ghi[:, k - ctl, :]
                        nc.tensor.matmul(aps[:], mt[:, k, :], g,
                                         start=(k == 0), stop=(k == ct - 1))

                    # agg psum -> sbuf bf16, then transpose
                    agg_sb = e_pool.tile([P, elem], DT.bfloat16, tag="agg")
                    if USE_SCALAR_ENG:
                        nc.scalar.copy(agg_sb[:], aps[:])
                    else:
                        nc.vector.tensor_copy(agg_sb[:], aps[:])
                    aggT = e_pool.tile([P, nh, P], DT.bfloat16, tag="aggT")
                    for h in range(nh):
                        if USE_VEC_TRANSPOSE:
                            nc.vector.transpose(
                                aggT[:, h, :], agg_sb[:, h * P:(h + 1) * P])
                        else:
                            tp = tr_psum.tile([P, P], DT.bfloat16)
                            nc.tensor.transpose(
                                tp[:], agg_sb[:, h * P:(h + 1) * P], ident[:])
                            nc.vector.tensor_copy(aggT[:, h, :], tp[:])

                    # dense layer + bias (ones-row chunk)
                    zps = z_psum.tile([P, HIDDEN], DT.float32, tag="zps")
                    for h in range(nh):
                        wslice = wp_sb[:] if l == 0 else wp_sb[:, h, :]
                        nc.tensor.matmul(zps[:], aggT[:, h, :], wslice,
                                         start=(h == 0), stop=False)
                    nc.tensor.matmul(zps[:], ones_sb[:], b_sb[l][:],
                                     start=False, stop=True)

                    if l < 2:
                        hsb = z_pool.tile([P, HIDDEN], DT.bfloat16, tag="h")
                        if USE_SCALAR_ENG:
                            nc.scalar.activation(hsb[:], zps[:], ACT.Relu)
                        else:
                            nc.vector.tensor_tensor(
                                out=hsb[:], in0=zps[:], in1=zero_sb[:],
                                op=AOP.max)
                        nc.sync.dma_start(
                            zslice[l][p * P:(p + 1) * P, :], hsb[:])
                        # fire AllGather chunk when its last tile completes
                        cidx = _chunk_of_pos(p)
                        if p == CHUNK_P0[cidx] + CHUNK_TILES[cidx] - 1:
                            r0 = CHUNK_P0[cidx] * P
                            r1 = r0 + CHUNK_TILES[cidx] * P
                            g0 = CHUNK_BASE[cidx]
                            g1_ = g0 + 8 * CHUNK_TILES[cidx] * P
                            nc.gpsimd.collective_compute(
                                "AllGather", AOP.bypass,
                                replica_groups=groups,
                                ins=[zslice[l][r0:r1, :]],
                                outs=[zfull[l][g0:g1_, :]])
                    else:
                        osb = z_pool.tile([P, HIDDEN], DT.float32, tag="o")
                        if USE_SCALAR_ENG:
                            nc.scalar.copy(osb[:], zps[:])
                        else:
                            nc.vector.tensor_copy(osb[:], zps[:])
                        nc.sync.dma_start(
                            out_d[p * P:(p + 1) * P, :], osb[:])
    nc.compile()
    return nc


# ---------------------------------------------------------------------------
# Entry point
# ---------------------------------------------------------------------------

LAST_RESULTS = None


def _run(cfg: Cfg, inputs: dict, trace: bool = False,
         trace_cores=None) -> np.ndarray:
    global LAST_RESULTS
    from concourse.bass_utils import run_bass_kernel_spmd

    in_maps, meta = _prep(cfg, **inputs)
    nc = _build(cfg, meta)
    kr = run_bass_kernel_spmd(nc, in_maps, list(range(cfg.ncores)), trace=trace,
                              trace_cores=trace_cores)
    LAST_RESULTS = kr
    res = kr.results
    out = np.empty((cfg.n, HIDDEN), dtype=np.float32)
    lrow = meta["lrow_of_node"]
    core = meta["core_of"]
    full = np.stack([res[c]["out"] for c in range(cfg.ncores)], axis=0)
    out[np.arange(cfg.n)] = full[core, lrow]
    return out


def kernel(**inputs) -> np.ndarray:
    return _run(CFG, inputs)
